# revision 17
# baseline (speedup 1.0000x reference)
"""BandSplit kernel for Trainium2 (8 NeuronCores, SPMD data-parallel over batch).

Reference computation (per band i, band width b, c=2b):
    xb[b,t,c]   = x[b, f0:f0+b, t, :] transposed/reshaped     (B, T, c)
    GroupNorm(1, c) over (T, c) per sample, affine gn_w/gn_b
    Linear(c -> 128) with fc_w/fc_b
    out stacked over 31 bands -> [B, T, 128, 31]

Key algebra: the whole band op is affine in x per sample:
    z[t,o] = s * sum_c x[t,c] * (gn_w[c]*fc_w[o,c])
             + (beta[o] + (-mu*s) * g[o])
  with s = rsqrt(var+eps), beta = fc_b + fc_w@gn_b, g = fc_w@gn_w.
The two bias terms enter the contraction through constant-1 activation
rows: one shared all-ones row carries every band's beta column block, and
one all-ones "g row" per band carries g scaled by (-mu*s).

Per-tile row layout (v2): [ones | g rows (1/band) | E rows | O rows];
everything outside the E/O blocks is constant 1.0, so the activation
tiles are memset once and only the E/O blocks are re-scattered per
sample (2 block DMAs per (xg tile, f tile) overlap = 14 per sample).

Weight columns within a matmul group are ordered (o, band) so the psum
drain writes runs of nb_g contiguous output words — the drain is the
only engine work on the output path and is split across the scalar,
vector, and gpsimd engines.
"""

import os
import numpy as np

import concourse.bass as bass
import concourse.tile as tile
import concourse.mybir as mybir
from concourse.bass_utils import run_bass_kernel_spmd

# ----------------------------------------------------------------------------
# Problem constants (hardcoded; kernel.py must be self-contained)
# ----------------------------------------------------------------------------
BANDS = [2, 3, 3, 3, 3, 3, 3, 3, 3, 3, 3, 8, 8, 8, 8, 8, 8, 8, 8, 8, 8, 8, 8,
         16, 16, 16, 16, 16, 16, 16, 17]
NB = len(BANDS)           # 31
CH = 128                  # output channels per band
EPS = 1e-5
B_FULL, F, T = 16, 257, 1000
N_CORES = 8
B_LOC = B_FULL // N_CORES  # 2 samples per core

# matmul input dtype: "f16" (1 cyc/col) or "f32" (4 cyc/col)
MM_DT = os.environ.get("BS_MM_DT", "f16")

# t-chunks of the main loop
CHUNKS = [(t0, min(128, T - t0)) for t0 in range(0, T, 128)]

# f-tiles of the raw input (aligned with band boundaries)
FT = [(0, 128), (128, 112), (240, 17)]
FT_BANDS = [(0, 23), (23, 30), (30, 31)]

# activation ("xg") tiles: bands packed so each tile stays <= 128 rows
TILE_BANDS = [(0, 13), (13, 20), (20, 25), (25, 28), (28, 31)]
N_XT = 5

# groups of <=4 bands per matmul (n = 128*nb <= 512 fits one psum bank)
GROUP_BANDS = [(0, 4), (4, 8), (8, 11), (11, 13),
               (13, 17), (17, 20),
               (20, 23), (23, 25),
               (25, 28),
               (28, 31)]
TILE_OF_GROUP = [0, 0, 0, 0, 1, 1, 2, 2, 3, 4]
# (psum_idx, col): psum tensor and column offset of each group's output.
# Adjacent same-width groups share a tensor at bank stride 512 so their
# drains merge into one instruction: {g0,g1} {g5,g6} {g8,g9}.
GROUP_PSUM = [(0, 0), (0, 512), (0, 1024), (1, 1024),
              (0, 1536), (1, 0), (1, 512), (1, 1536),
              (2, 0), (2, 512)]
# merged drain schedule: (pi, kind, col0, nb, blo, engine); kind "pair"
# drains two groups at bank stride 512 covering 2*nb adjacent bands
DRAINS = [(0, "pair", 0, 4, 0, "act"),      # g0+g1  bands 0-7
          (0, "one", 1024, 3, 8, "dve"),    # g2     bands 8-10
          (0, "one", 1536, 4, 13, "dve"),   # g4     bands 13-16
          (1, "pair", 0, 3, 17, "dve"),     # g5+g6  bands 17-22
          (1, "one", 1024, 2, 11, "act"),   # g3     bands 11-12
          (1, "one", 1536, 2, 23, "dve"),   # g7     bands 23-24
          (2, "pair", 0, 3, 25, "act")]     # g8+g9  bands 25-30


def _tile_geom():
    """Per-tile row geometry: (lo, hi, nb, SB, EST, OST, R)."""
    geom = []
    for (lo, hi) in TILE_BANDS:
        nb = hi - lo
        sb = sum(BANDS[lo:hi])
        est = 1 + nb
        ost = est + sb
        geom.append((lo, hi, nb, sb, est, ost, ost + sb))
    return geom

GEOM = _tile_geom()
TILE_ROWS = [g[6] for g in GEOM]

# wt/p1 column layout: per tile, groups concatenated; within a group the
# column order is (o, band_in_group)
WT_COLS = [g[2] * CH for g in GEOM]                       # nb_t * 128
WT_OFF = [sum(WT_COLS[:t]) for t in range(N_XT)]
TOT_COLS = sum(WT_COLS)                                   # 3968
GWOFF = []                                                # group -> local col
_acc = {}
for _g, (_blo, _bhi) in enumerate(GROUP_BANDS):
    _t = TILE_OF_GROUP[_g]
    GWOFF.append(_acc.get(_t, 0))
    _acc[_t] = GWOFF[-1] + (_bhi - _blo) * CH

# rend per group: rows [0, rend) of the tile participate in the matmul
REND = []
for _g, (_blo, _bhi) in enumerate(GROUP_BANDS):
    _t = TILE_OF_GROUP[_g]
    lo, hi, nb, sb, est, ost, rr = GEOM[_t]
    REND.append(ost + sum(BANDS[lo:_bhi]))

# E/O scatter blocks: (ft, t, src_row0, dst_E, dst_O, nrows)
SCATTER = []
for _ft, (_b0, _b1) in enumerate(FT_BANDS):
    for _t, (_lo, _hi) in enumerate(TILE_BANDS):
        ov_lo, ov_hi = max(_b0, _lo), min(_b1, _hi)
        if ov_lo >= ov_hi:
            continue
        src0 = sum(BANDS[_b0:ov_lo])
        nrows = sum(BANDS[ov_lo:ov_hi])
        lo, hi, nb, sb, est, ost, rr = GEOM[_t]
        off = sum(BANDS[_lo:ov_lo])
        SCATTER.append((_ft, _t, src0, est + off, ost + off, nrows))


def _build_const_tables(gn_w, gn_b, fc_w, fc_b):
    """Host-side packing of the (tiny) parameters into matmul-ready tables."""
    p1 = np.zeros((128, TOT_COLS), np.float32)
    msel = np.zeros((63, N_XT * 128), np.float32)
    for g, (blo, bhi) in enumerate(GROUP_BANDS):
        t = TILE_OF_GROUP[g]
        lo, hi, nb_t, sb, est, ost, rr = GEOM[t]
        nb_g = bhi - blo
        base = WT_OFF[t] + GWOFF[g]
        for j, i in enumerate(range(blo, bhi)):
            b = BANDS[i]
            c = 2 * b
            w = fc_w[i, :, :c].astype(np.float64)          # [128, c]
            beta = fc_b[i] + w @ gn_b[i, :c]               # [128]
            gv = w @ gn_w[i, :c]                           # [128]
            w2 = (w * gn_w[i, :c][None, :]).T              # [c, 128]
            cols = base + np.arange(CH) * nb_g + j
            p1[0, cols] = beta
            p1[1 + (i - lo), cols] = gv
            cumb = sum(BANDS[lo:i])
            for k in range(b):
                p1[est + cumb + k, cols] = w2[2 * k]       # E row
                p1[ost + cumb + k, cols] = w2[2 * k + 1]   # O row
    # msel: [63, N_XT*128]; csb_col(t) = msel[:, t*128:(t+1)*128]^T @ vec63
    # vec63 = [s_0..s_30, (-mu*s)_0..30, 1.0]
    for t, (lo, hi, nb_t, sb, est, ost, rr) in enumerate(GEOM):
        col = t * 128
        msel[62, col + 0] = 1.0                            # ones row: C=1
        for i in range(lo, hi):
            msel[31 + i, col + 1 + (i - lo)] = 1.0         # g row: C=-mu*s
            cumb = sum(BANDS[lo:i])
            b = BANDS[i]
            msel[i, col + est + cumb: col + est + cumb + b] = 1.0
            msel[i, col + ost + cumb: col + ost + cumb + b] = 1.0

    # Ind: [257, 31] band indicator over f rows
    ind = np.zeros((F, NB), np.float32)
    f0 = 0
    for i, b in enumerate(BANDS):
        ind[f0:f0 + b, i] = 1.0
        f0 += b

    # invCT2: [1, 62] = 1 / (c_i * T), duplicated for the Sx and Sxx halves
    invct = np.array([1.0 / (2 * b * T) for b in BANDS], np.float32)
    invct2 = np.concatenate([invct, invct])[None, :]
    return p1, msel, ind, invct2


# ----------------------------------------------------------------------------
# Bass kernel
# ----------------------------------------------------------------------------
_NC_CACHE = {}


def _spill_waits(nc):
    """Split multi-wait instructions into NoOp(wait) + instruction.

    The walrus build in this container enforces the HW wait capacity
    (1 sync wait per instruction, 2 for EventSemaphore); Tile emits more.
    Engine queues are in-order, so hoisting extra waits into preceding
    NoOps on the same queue preserves semantics.
    """
    n = 0
    for fn in nc.m.functions:
        for bb in fn.blocks:
            out = []
            changed = False
            for inst in bb.instructions:
                si = getattr(inst, "sync_info", None)
                cap = 2 if isinstance(inst, mybir.InstEventSemaphore) else 1
                if si is not None and si.on_wait and len(si.on_wait) > cap:
                    waits = list(si.on_wait)
                    extra, keep = waits[:-cap], waits[-cap:]
                    for w in extra:
                        nop = mybir.InstNoOp(name=f"{inst.name}_w{n}",
                                             ins=[], outs=[])
                        nop.engine = inst.engine
                        nop.sync_info = mybir.SyncInfo(on_wait=[w],
                                                       on_update=[])
                        out.append(nop)
                        n += 1
                    si.on_wait = keep
                    changed = True
                out.append(inst)
            if changed:
                bb.instructions = out
    return n


def _mm_dt():
    return mybir.dt.float32 if MM_DT == "f32" else mybir.dt.float16


def build_bass():
    repeat = int(os.environ.get("BS_REPEAT", "1"))
    key = (MM_DT, repeat)
    if key in _NC_CACHE:
        return _NC_CACHE[key]
    DT = _mm_dt()
    F32 = mybir.dt.float32

    nc = bass.Bass("TRN2", target_bir_lowering=False, debug=False,
                   num_devices=N_CORES)

    x_d = nc.dram_tensor("x", [B_LOC, F, T, 2], DT, kind="ExternalInput").ap()
    p1_d = nc.dram_tensor("p1", [128, TOT_COLS], DT, kind="ExternalInput").ap()
    msel_d = nc.dram_tensor("msel", [63, N_XT * 128], F32, kind="ExternalInput").ap()
    ind_d = nc.dram_tensor("ind", [F, NB], F32, kind="ExternalInput").ap()
    invct_d = nc.dram_tensor("invct2", [1, 2 * NB], F32, kind="ExternalInput").ap()
    ones_d = nc.dram_tensor("ones16", [14, B_LOC * N_XT * T], DT,
                            kind="ExternalInput").ap()
    z_d = nc.dram_tensor("z", [B_LOC, T, CH, NB], F32, kind="ExternalOutput").ap()

    AluOp = mybir.AluOpType
    ActFn = mybir.ActivationFunctionType

    with tile.TileContext(nc) as tc:
        with (
            tc.tile_pool(name="const", bufs=1) as constp,
            tc.tile_pool(name="a", bufs=6) as ap_,
            tc.tile_pool(name="eo", bufs=6) as eop,
            tc.tile_pool(name="xg", bufs=1) as xgp,
            tc.tile_pool(name="wt", bufs=1) as wtp,
            tc.tile_pool(name="small", bufs=8) as smp,
            tc.tile_pool(name="out", bufs=3) as outp,
            tc.tile_pool(name="sq", bufs=1) as sqp,
            tc.tile_pool(name="psum", bufs=2, space="PSUM") as psp,
        ):
            # ---------------- constants to SBUF ----------------
            p1_sb = constp.tile([128, TOT_COLS], DT, tag="p1",
                                name="p1c")
            msel_sb = constp.tile([63, N_XT * 128], F32, tag="msel")
            nc.sync.dma_start(msel_sb[:], msel_d[:])
            ind_sb = []
            for g, (f0, P) in enumerate(FT):
                it = constp.tile([P, NB], F32, tag=f"ind_{g}", name=f"indc_{g}")
                nc.sync.dma_start(it[:], ind_d[f0:f0 + P, :])
                ind_sb.append(it)
            invct_sb = constp.tile([1, 2 * NB], F32, tag="invct")
            nc.sync.dma_start(invct_sb[:], invct_d[:])
            ident = constp.tile([1, 1], F32, tag="ident")
            nc.vector.memset(ident[:], 1.0)
            zcol = constp.tile([128, 1], F32, tag="zcol")
            nc.vector.memset(zcol[:], 0.0)
            epsc = constp.tile([1, 1], F32, tag="epsc")
            nc.vector.memset(epsc[:], EPS)

            # persistent activation / weight tensors.  xg is one wide
            # tensor (tile t of sample s at cols (s*N_XT+t)*T) so the
            # constant ones+g rows load with a single DMA; E/O blocks are
            # re-scattered per sample.
            xg_all = xgp.tile([128, B_LOC * N_XT * T], DT, tag="xg",
                              name="xg_all")
            xg_base = [[(s * N_XT + t) * T for t in range(N_XT)]
                       for s in range(B_LOC)]
            wt = [wtp.tile([128, TOT_COLS], DT, tag=f"wt_{s}",
                           name=f"wt_{s}") for s in range(B_LOC)]
            nc.gpsimd.dma_start(xg_all[0:14, :], ones_d[:])

            # ---------------- body (repeatable for benchmarking) ------
            n_chunks = int(os.environ.get("BS_NCHUNKS", "8"))
            skip_mm = os.environ.get("BS_SKIP_MM") == "1"
            skip_drain = os.environ.get("BS_SKIP_DRAIN") == "1"
            skip_out = os.environ.get("BS_SKIP_OUT") == "1"

            def loads(s, queue=None):
                """Input loads for sample s (default: sync DMA queue)."""
                q = queue if queue is not None else nc.sync
                As = []
                for g, (f0, P) in enumerate(FT):
                    A = ap_.tile([P, 2000], DT, tag="a", name=f"A_{s}_{g}")
                    q.dma_start(
                        A[:], x_d[s, f0:f0 + P].rearrange("p a b -> p (a b)"))
                    As.append(A)
                return As

            def stats_pre(s, As):
                """square+dei+row sums, E/O scatter. No PE ops."""
                stats = []
                for g, (f0, P) in enumerate(FT):
                    A = As[g]
                    stat = smp.tile([P, 2], F32, tag="stat",
                                    name=f"stat_{s}_{g}")
                    s1t = smp.tile([P, 2], F32, tag="s1t", name=f"s1t_{s}_{g}")
                    # fused square + per-row sum on the scalar engine
                    Asq = sqp.tile([P, 2000], DT, tag="sq", name="Asq")
                    nc.scalar.activation(Asq[:], A[:], ActFn.Square,
                                         bias=zcol[0:P, :],
                                         accum_out=stat[:, 1:2])
                    # de-interleave (and cast); accumulate sums per f-row
                    Av = A[:].rearrange("p (t r) -> p r t", r=2)
                    E = eop.tile([P, T], DT, tag="eo", name=f"E_{s}_{g}")
                    O = eop.tile([P, T], DT, tag="eo", name=f"O_{s}_{g}")
                    nc.vector.tensor_scalar(E[:], Av[:, 0, :], 1.0, None,
                                            AluOp.mult, AluOp.add,
                                            accum_out=s1t[:, 0:1])
                    nc.vector.tensor_scalar(O[:], Av[:, 1, :], 1.0, None,
                                            AluOp.mult, AluOp.add,
                                            accum_out=s1t[:, 1:2])
                    nc.vector.tensor_tensor(stat[:, 0:1], s1t[:, 0:1],
                                            s1t[:, 1:2], AluOp.add)
                    stats.append(stat)
                    # block-scatter E/O rows into the xg strip tiles
                    for (ft, t, src0, dstE, dstO, nr) in SCATTER:
                        if ft != g:
                            continue
                        b0 = xg_base[s][t]
                        nc.gpsimd.dma_start(
                            xg_all[dstE:dstE + nr, b0:b0 + T],
                            E[src0:src0 + nr, :])
                        nc.gpsimd.dma_start(
                            xg_all[dstO:dstO + nr, b0:b0 + T],
                            O[src0:src0 + nr, :])
                return stats

            def weights(s, stats):
                """Per-band moments -> scale vector -> scaled weight tables."""
                mom = psp.tile([1, 2 * NB], F32, tag="main", name=f"mom_{s}")
                for g in range(len(FT)):
                    # partial sums -> mom[0, b0:b1] (Sx), [NB+b0:NB+b1] (Sxx)
                    b0, b1 = FT_BANDS[g]
                    nc.tensor.matmul(mom[0:1, b0:b1], lhsT=stats[g][:, 0:1],
                                     rhs=ind_sb[g][:, b0:b1],
                                     start=True, stop=True)
                    nc.tensor.matmul(mom[0:1, NB + b0:NB + b1],
                                     lhsT=stats[g][:, 1:2],
                                     rhs=ind_sb[g][:, b0:b1],
                                     start=True, stop=True)
                # moments -> s, -mu*s (everything on partition 0, free axis)
                m2 = smp.tile([1, 2 * NB], F32, tag="m2")
                nc.vector.tensor_tensor(m2[:], mom[:], invct_sb[:],
                                        AluOp.mult)   # [mu | ex2]
                mu = m2[:, 0:NB]
                ex2 = m2[:, NB:2 * NB]
                var = smp.tile([1, NB], F32, tag="var")
                nc.vector.tensor_tensor(var[:], mu, mu, AluOp.mult)  # mu^2
                nc.vector.tensor_tensor(var[:], ex2, var[:],
                                        AluOp.subtract)   # ex2 - mu^2
                sd = smp.tile([1, NB], F32, tag="sd")
                nc.scalar.activation(sd[:], var[:], ActFn.Sqrt,
                                     bias=epsc[:])
                vrow = smp.tile([1, 64], F32, tag="vrow")
                nc.vector.reciprocal(vrow[:, 0:NB], sd[:])         # s
                tmp = smp.tile([1, NB], F32, tag="tmp")
                nc.vector.tensor_tensor(tmp[:], mu, vrow[:, 0:NB],
                                        AluOp.mult)       # mu*s
                nc.vector.tensor_scalar(vrow[:, NB:2 * NB], tmp[:], -1.0, None,
                                        AluOp.mult)       # -mu*s
                nc.vector.memset(vrow[:, 62:63], 1.0)

                v63p = psp.tile([63, 1], F32, tag="main", name=f"v63p_{s}")
                nc.tensor.transpose(v63p[:], vrow[:, 0:63], ident[:])
                v63 = smp.tile([63, 1], F32, tag="v63")
                nc.vector.tensor_copy(v63[:], v63p[:])

                cvp = psp.tile([128, N_XT], F32, tag="main", name=f"cvp_{s}")
                for t in range(N_XT):
                    nc.tensor.matmul(cvp[:, t:t + 1],
                                     lhsT=msel_sb[:, t * 128:(t + 1) * 128],
                                     rhs=v63[:], start=True, stop=True)
                csb = smp.tile([128, N_XT], F32, tag="csb", name=f"csb_{s}")
                nc.vector.tensor_copy(csb[:], cvp[:])
                for t in range(N_XT):
                    c0, c1 = WT_OFF[t], WT_OFF[t] + WT_COLS[t]
                    nc.scalar.activation(wt[s][:, c0:c1], p1_sb[:, c0:c1],
                                         ActFn.Copy,
                                         scale=csb[:, t:t + 1])

            def chunk(s, t0, M):
                ob = outp.tile([128, CH * NB], F32, tag="ob", name="ob")
                # ob free index = o*31 + i  (the DRAM layout)
                ob_v = ob[0:M].rearrange("p (o i) -> p o i", o=CH, i=NB)
                for pi in range(3):
                    pt = psp.tile([128, 2048], F32, tag="main",
                                  name=f"ps{pi}")
                    for g, (blo, bhi) in enumerate(GROUP_BANDS):
                        if GROUP_PSUM[g][0] != pi:
                            continue
                        col = GROUP_PSUM[g][1]
                        t = TILE_OF_GROUP[g]
                        nb_g = bhi - blo
                        n = nb_g * CH
                        rend = REND[g]
                        gw0 = GWOFF[g]
                        if not skip_mm:
                            xb = xg_base[s][t]
                            wb = WT_OFF[t] + gw0
                            nc.tensor.matmul(
                                pt[0:M, col:col + n],
                                lhsT=xg_all[0:rend, xb + t0:xb + t0 + M],
                                rhs=wt[s][0:rend, wb:wb + n],
                                start=True, stop=True)
                    # drains for this psum tensor (split across engines)
                    if skip_drain:
                        continue
                    for (dpi, kind, col, nb_g, blo, eng) in DRAINS:
                        if dpi != pi:
                            continue
                        if kind == "pair":
                            gi = col // 512
                            src = pt[0:M].rearrange(
                                "p (g r) -> p g r", g=4, r=512)[
                                :, gi:gi + 2, 0:nb_g * CH].rearrange(
                                "p g (o i) -> p o g i", o=CH, i=nb_g)
                            dst = ob_v[:, :, blo:blo + 2 * nb_g].rearrange(
                                "p o (g i) -> p o g i", g=2, i=nb_g)
                        else:
                            src = pt[0:M, col:col + nb_g * CH].rearrange(
                                "p (o i) -> p o i", o=CH, i=nb_g)
                            dst = ob_v[:, :, blo:blo + nb_g]
                        if eng == "act":
                            nc.scalar.copy(dst, src)
                        else:
                            nc.vector.tensor_copy(dst, src)
                if not skip_out:
                    nc.sync.dma_start(
                        z_d[s, t0:t0 + M].rearrange("p a b -> p (a b)"),
                        ob[0:M, :])

            for _rep in range(repeat):
                # interleaved emission: every engine queue is ordered by
                # expected data-ready time so in-order queues never block
                # early-ready work behind late-ready work.  DMA priority on
                # the sync queue: A(s0) -> p1 -> A(s1) -> out stream.
                A0 = loads(0)
                if _rep == 0:
                    nc.sync.dma_start(p1_sb[:], p1_d[:])
                A1 = loads(1)
                st0 = stats_pre(0, A0)
                weights(0, st0)
                chunk(0, *CHUNKS[0])
                st1 = stats_pre(1, A1)
                weights(1, st1)
                for (t0, M) in CHUNKS[1:n_chunks]:
                    chunk(0, t0, M)
                for (t0, M) in CHUNKS[:n_chunks]:
                    chunk(1, t0, M)

    _NC_CACHE[key] = nc
    return nc


# ----------------------------------------------------------------------------
# Public entry point
# ----------------------------------------------------------------------------
def kernel(x, gn_w, gn_b, fc_w, fc_b):
    x = np.asarray(x, np.float32)
    gn_w = np.asarray(gn_w, np.float32)
    gn_b = np.asarray(gn_b, np.float32)
    fc_w = np.asarray(fc_w, np.float32)
    fc_b = np.asarray(fc_b, np.float32)

    p1, msel, ind, invct2 = _build_const_tables(gn_w, gn_b, fc_w, fc_b)
    np_dt = np.float16 if MM_DT == "f16" else np.float32
    ones16 = np.ones((14, B_LOC * N_XT * T), np_dt)
    p1 = p1.astype(np_dt)
    x = x.astype(np_dt)
    nc = build_bass()
    if not getattr(nc, "_waits_spilled", False):
        _spill_waits(nc)
        nc._waits_spilled = True

    in_maps = []
    for k in range(N_CORES):
        in_maps.append({
            "x": np.ascontiguousarray(x[k * B_LOC:(k + 1) * B_LOC]),
            "p1": p1, "msel": msel, "ind": ind,
            "invct2": invct2, "ones16": ones16,
        })
    res = run_bass_kernel_spmd(nc, in_maps, core_ids=list(range(N_CORES)))
    z = np.concatenate([r["z"] for r in res.results], axis=0)
    return z


# revision 18
# speedup vs baseline: 1.1837x; 1.1837x over previous
"""BandSplit kernel for Trainium2 (8 NeuronCores, SPMD data-parallel over batch).

Reference computation (per band i, band width b, c=2b):
    xb[b,t,c]   = x[b, f0:f0+b, t, :] transposed/reshaped     (B, T, c)
    GroupNorm(1, c) over (T, c) per sample, affine gn_w/gn_b
    Linear(c -> 128) with fc_w/fc_b
    out stacked over 31 bands -> [B, T, 128, 31]

Key algebra: the whole band op is affine in x per sample:
    z[t,o] = s * sum_c x[t,c] * (gn_w[c]*fc_w[o,c])
             + (beta[o] + (-mu*s) * g[o])
  with s = rsqrt(var+eps), beta = fc_b + fc_w@gn_b, g = fc_w@gn_w.
The two bias terms enter the contraction through constant-1 activation
rows: one shared all-ones row carries every band's beta column block, and
one all-ones "g row" per band carries g scaled by (-mu*s).

Per-tile row layout (v2): [ones | g rows (1/band) | E rows | O rows];
everything outside the E/O blocks is constant 1.0, so the activation
tiles are memset once and only the E/O blocks are re-scattered per
sample (2 block DMAs per (xg tile, f tile) overlap = 14 per sample).

Weight columns within a matmul group are ordered (o, band) so the psum
drain writes runs of nb_g contiguous output words — the drain is the
only engine work on the output path and is split across the scalar,
vector, and gpsimd engines.
"""

import os
import numpy as np

import concourse.bass as bass
import concourse.tile as tile
import concourse.mybir as mybir
from concourse.bass_utils import run_bass_kernel_spmd

# ----------------------------------------------------------------------------
# Problem constants (hardcoded; kernel.py must be self-contained)
# ----------------------------------------------------------------------------
BANDS = [2, 3, 3, 3, 3, 3, 3, 3, 3, 3, 3, 8, 8, 8, 8, 8, 8, 8, 8, 8, 8, 8, 8,
         16, 16, 16, 16, 16, 16, 16, 17]
NB = len(BANDS)           # 31
CH = 128                  # output channels per band
EPS = 1e-5
B_FULL, F, T = 16, 257, 1000
N_CORES = 8
B_LOC = B_FULL // N_CORES  # 2 samples per core

# matmul input dtype: "f16" (1 cyc/col) or "f32" (4 cyc/col)
MM_DT = os.environ.get("BS_MM_DT", "f16")

# t-chunks of the main loop
CHUNKS = [(t0, min(128, T - t0)) for t0 in range(0, T, 128)]

# f-tiles of the raw input (aligned with band boundaries)
FT = [(0, 128), (128, 112), (240, 17)]
FT_BANDS = [(0, 23), (23, 30), (30, 31)]

# activation ("xg") tiles: bands packed so each tile stays <= 128 rows
TILE_BANDS = [(0, 13), (13, 20), (20, 25), (25, 28), (28, 31)]
N_XT = 5

# groups of <=4 bands per matmul (n = 128*nb <= 512 fits one psum bank)
GROUP_BANDS = [(0, 4), (4, 8), (8, 11), (11, 13),
               (13, 17), (17, 20),
               (20, 23), (23, 25),
               (25, 28),
               (28, 31)]
TILE_OF_GROUP = [0, 0, 0, 0, 1, 1, 2, 2, 3, 4]
# (psum_idx, col): psum tensor and column offset of each group's output.
# Adjacent same-width groups share a tensor at bank stride 512 so their
# drains merge into one instruction: {g0,g1} {g5,g6} {g8,g9}.
GROUP_PSUM = [(0, 0), (0, 512), (0, 1024), (1, 1024),
              (0, 1536), (1, 0), (1, 512), (1, 1536),
              (2, 0), (2, 512)]
# merged drain schedule: (pi, kind, col0, nb, blo, engine); kind "pair"
# drains two groups at bank stride 512 covering 2*nb adjacent bands
DRAINS = [(0, "pair", 0, 4, 0, "act"),      # g0+g1  bands 0-7
          (0, "one", 1024, 3, 8, "dve"),    # g2     bands 8-10
          (0, "one", 1536, 4, 13, "dve"),   # g4     bands 13-16
          (1, "pair", 0, 3, 17, "dve"),     # g5+g6  bands 17-22
          (1, "one", 1024, 2, 11, "act"),   # g3     bands 11-12
          (1, "one", 1536, 2, 23, "dve"),   # g7     bands 23-24
          (2, "pair", 0, 3, 25, "act")]     # g8+g9  bands 25-30


def _tile_geom():
    """Per-tile row geometry: (lo, hi, nb, SB, EST, OST, R)."""
    geom = []
    for (lo, hi) in TILE_BANDS:
        nb = hi - lo
        sb = sum(BANDS[lo:hi])
        est = 1 + nb
        ost = est + sb
        geom.append((lo, hi, nb, sb, est, ost, ost + sb))
    return geom

GEOM = _tile_geom()
TILE_ROWS = [g[6] for g in GEOM]

# wt/p1 column layout: per tile, groups concatenated; within a group the
# column order is (o, band_in_group)
WT_COLS = [g[2] * CH for g in GEOM]                       # nb_t * 128
WT_OFF = [sum(WT_COLS[:t]) for t in range(N_XT)]
TOT_COLS = sum(WT_COLS)                                   # 3968
GWOFF = []                                                # group -> local col
_acc = {}
for _g, (_blo, _bhi) in enumerate(GROUP_BANDS):
    _t = TILE_OF_GROUP[_g]
    GWOFF.append(_acc.get(_t, 0))
    _acc[_t] = GWOFF[-1] + (_bhi - _blo) * CH

# rend per group: rows [0, rend) of the tile participate in the matmul
REND = []
for _g, (_blo, _bhi) in enumerate(GROUP_BANDS):
    _t = TILE_OF_GROUP[_g]
    lo, hi, nb, sb, est, ost, rr = GEOM[_t]
    REND.append(ost + sum(BANDS[lo:_bhi]))

# E/O scatter blocks: (ft, t, src_row0, dst_E, dst_O, nrows)
SCATTER = []
for _ft, (_b0, _b1) in enumerate(FT_BANDS):
    for _t, (_lo, _hi) in enumerate(TILE_BANDS):
        ov_lo, ov_hi = max(_b0, _lo), min(_b1, _hi)
        if ov_lo >= ov_hi:
            continue
        src0 = sum(BANDS[_b0:ov_lo])
        nrows = sum(BANDS[ov_lo:ov_hi])
        lo, hi, nb, sb, est, ost, rr = GEOM[_t]
        off = sum(BANDS[_lo:ov_lo])
        SCATTER.append((_ft, _t, src0, est + off, ost + off, nrows))


def _build_const_tables(gn_w, gn_b, fc_w, fc_b):
    """Host-side packing of the (tiny) parameters into matmul-ready tables."""
    p1 = np.zeros((128, TOT_COLS), np.float32)
    msel = np.zeros((63, N_XT * 128), np.float32)
    for g, (blo, bhi) in enumerate(GROUP_BANDS):
        t = TILE_OF_GROUP[g]
        lo, hi, nb_t, sb, est, ost, rr = GEOM[t]
        nb_g = bhi - blo
        base = WT_OFF[t] + GWOFF[g]
        for j, i in enumerate(range(blo, bhi)):
            b = BANDS[i]
            c = 2 * b
            w = fc_w[i, :, :c].astype(np.float64)          # [128, c]
            beta = fc_b[i] + w @ gn_b[i, :c]               # [128]
            gv = w @ gn_w[i, :c]                           # [128]
            w2 = (w * gn_w[i, :c][None, :]).T              # [c, 128]
            cols = base + np.arange(CH) * nb_g + j
            p1[0, cols] = beta
            p1[1 + (i - lo), cols] = gv
            cumb = sum(BANDS[lo:i])
            for k in range(b):
                p1[est + cumb + k, cols] = w2[2 * k]       # E row
                p1[ost + cumb + k, cols] = w2[2 * k + 1]   # O row
    # msel: [63, N_XT*128]; csb_col(t) = msel[:, t*128:(t+1)*128]^T @ vec63
    # vec63 = [s_0..s_30, (-mu*s)_0..30, 1.0]
    for t, (lo, hi, nb_t, sb, est, ost, rr) in enumerate(GEOM):
        col = t * 128
        msel[62, col + 0] = 1.0                            # ones row: C=1
        for i in range(lo, hi):
            msel[31 + i, col + 1 + (i - lo)] = 1.0         # g row: C=-mu*s
            cumb = sum(BANDS[lo:i])
            b = BANDS[i]
            msel[i, col + est + cumb: col + est + cumb + b] = 1.0
            msel[i, col + ost + cumb: col + ost + cumb + b] = 1.0

    # Ind: [257, 31] band indicator over f rows
    ind = np.zeros((F, NB), np.float32)
    f0 = 0
    for i, b in enumerate(BANDS):
        ind[f0:f0 + b, i] = 1.0
        f0 += b

    # invCT2: [1, 62] = 1 / (c_i * T), duplicated for the Sx and Sxx halves
    invct = np.array([1.0 / (2 * b * T) for b in BANDS], np.float32)
    invct2 = np.concatenate([invct, invct])[None, :]
    return p1, msel, ind, invct2


# ----------------------------------------------------------------------------
# Bass kernel
# ----------------------------------------------------------------------------
_NC_CACHE = {}


def _spill_waits(nc):
    """Split multi-wait instructions into NoOp(wait) + instruction.

    The walrus build in this container enforces the HW wait capacity
    (1 sync wait per instruction, 2 for EventSemaphore); Tile emits more.
    Engine queues are in-order, so hoisting extra waits into preceding
    NoOps on the same queue preserves semantics.
    """
    n = 0
    for fn in nc.m.functions:
        for bb in fn.blocks:
            out = []
            changed = False
            for inst in bb.instructions:
                si = getattr(inst, "sync_info", None)
                cap = 2 if isinstance(inst, mybir.InstEventSemaphore) else 1
                if si is not None and si.on_wait and len(si.on_wait) > cap:
                    waits = list(si.on_wait)
                    extra, keep = waits[:-cap], waits[-cap:]
                    for w in extra:
                        nop = mybir.InstNoOp(name=f"{inst.name}_w{n}",
                                             ins=[], outs=[])
                        nop.engine = inst.engine
                        nop.sync_info = mybir.SyncInfo(on_wait=[w],
                                                       on_update=[])
                        out.append(nop)
                        n += 1
                    si.on_wait = keep
                    changed = True
                out.append(inst)
            if changed:
                bb.instructions = out
    return n


def _mm_dt():
    return mybir.dt.float32 if MM_DT == "f32" else mybir.dt.float16


def build_bass():
    repeat = int(os.environ.get("BS_REPEAT", "1"))
    key = (MM_DT, repeat)
    if key in _NC_CACHE:
        return _NC_CACHE[key]
    DT = _mm_dt()
    F32 = mybir.dt.float32

    nc = bass.Bass("TRN2", target_bir_lowering=False, debug=False,
                   num_devices=N_CORES)

    x_d = nc.dram_tensor("x", [B_LOC, F, T, 2], DT, kind="ExternalInput").ap()
    p1_d = nc.dram_tensor("p1", [128, TOT_COLS], DT, kind="ExternalInput").ap()
    msel_d = nc.dram_tensor("msel", [63, N_XT * 128], F32, kind="ExternalInput").ap()
    ind_d = nc.dram_tensor("ind", [F, NB], F32, kind="ExternalInput").ap()
    invct_d = nc.dram_tensor("invct2", [1, 2 * NB], F32, kind="ExternalInput").ap()
    ones_d = nc.dram_tensor("ones16", [14, B_LOC * N_XT * T], DT,
                            kind="ExternalInput").ap()
    z_d = nc.dram_tensor("z", [B_LOC, T, CH, NB], F32, kind="ExternalOutput").ap()

    AluOp = mybir.AluOpType
    ActFn = mybir.ActivationFunctionType

    with tile.TileContext(nc) as tc:
        with (
            tc.tile_pool(name="const", bufs=1) as constp,
            tc.tile_pool(name="a", bufs=6) as ap_,
            tc.tile_pool(name="eo", bufs=6) as eop,
            tc.tile_pool(name="xg", bufs=1) as xgp,
            tc.tile_pool(name="wt", bufs=1) as wtp,
            tc.tile_pool(name="small", bufs=8) as smp,
            tc.tile_pool(name="out", bufs=3) as outp,
            tc.tile_pool(name="sq", bufs=1) as sqp,
            tc.tile_pool(name="psum", bufs=2, space="PSUM") as psp,
        ):
            # ---------------- constants to SBUF ----------------
            p1_sb = constp.tile([128, TOT_COLS], DT, tag="p1",
                                name="p1c")
            msel_sb = constp.tile([63, N_XT * 128], F32, tag="msel")
            nc.sync.dma_start(msel_sb[:], msel_d[:])
            ind_sb = []
            for g, (f0, P) in enumerate(FT):
                it = constp.tile([P, NB], F32, tag=f"ind_{g}", name=f"indc_{g}")
                nc.sync.dma_start(it[:], ind_d[f0:f0 + P, :])
                ind_sb.append(it)
            invct_sb = constp.tile([1, 2 * NB], F32, tag="invct")
            nc.sync.dma_start(invct_sb[:], invct_d[:])
            ident = constp.tile([1, 1], F32, tag="ident")
            nc.vector.memset(ident[:], 1.0)
            zcol = constp.tile([128, 1], F32, tag="zcol")
            nc.vector.memset(zcol[:], 0.0)
            epsc = constp.tile([1, 1], F32, tag="epsc")
            nc.vector.memset(epsc[:], EPS)

            # persistent per-(sample,tile) tensors — separate tensors keep
            # the tile framework's dependency tracking fine-grained.  The
            # constant ones+g rows load once from DRAM; E/O blocks are
            # re-scattered per sample.
            xg = [[xgp.tile([128, T], DT, tag=f"xg_{s}_{t}", name=f"xg_{s}_{t}")
                   for t in range(N_XT)] for s in range(B_LOC)]
            wt = [[wtp.tile([128, WT_COLS[t]], DT, tag=f"wt_{s}_{t}",
                            name=f"wt_{s}_{t}")
                   for t in range(N_XT)] for s in range(B_LOC)]
            for s in range(B_LOC):
                for t in range(N_XT):
                    est = GEOM[t][4]
                    nc.gpsimd.dma_start(xg[s][t][0:est, :],
                                        ones_d[0:est, 0:T])

            # ---------------- body (repeatable for benchmarking) ------
            n_chunks = int(os.environ.get("BS_NCHUNKS", "8"))
            skip_mm = os.environ.get("BS_SKIP_MM") == "1"
            skip_drain = os.environ.get("BS_SKIP_DRAIN") == "1"
            skip_out = os.environ.get("BS_SKIP_OUT") == "1"

            def loads(s, queue=None):
                """Input loads for sample s (default: sync DMA queue)."""
                q = queue if queue is not None else nc.sync
                As = []
                for g, (f0, P) in enumerate(FT):
                    A = ap_.tile([P, 2000], DT, tag="a", name=f"A_{s}_{g}")
                    q.dma_start(
                        A[:], x_d[s, f0:f0 + P].rearrange("p a b -> p (a b)"))
                    As.append(A)
                return As

            def stats_pre(s, As):
                """square+dei+row sums, E/O scatter. No PE ops."""
                stats = []
                for g, (f0, P) in enumerate(FT):
                    A = As[g]
                    stat = smp.tile([P, 2], F32, tag="stat",
                                    name=f"stat_{s}_{g}")
                    s1t = smp.tile([P, 2], F32, tag="s1t", name=f"s1t_{s}_{g}")
                    # fused square + per-row sum on the scalar engine
                    Asq = sqp.tile([P, 2000], DT, tag="sq", name="Asq")
                    nc.scalar.activation(Asq[:], A[:], ActFn.Square,
                                         bias=zcol[0:P, :],
                                         accum_out=stat[:, 1:2])
                    # de-interleave (and cast); accumulate sums per f-row
                    Av = A[:].rearrange("p (t r) -> p r t", r=2)
                    E = eop.tile([P, T], DT, tag="eo", name=f"E_{s}_{g}")
                    O = eop.tile([P, T], DT, tag="eo", name=f"O_{s}_{g}")
                    nc.vector.tensor_scalar(E[:], Av[:, 0, :], 1.0, None,
                                            AluOp.mult, AluOp.add,
                                            accum_out=s1t[:, 0:1])
                    nc.vector.tensor_scalar(O[:], Av[:, 1, :], 1.0, None,
                                            AluOp.mult, AluOp.add,
                                            accum_out=s1t[:, 1:2])
                    nc.vector.tensor_tensor(stat[:, 0:1], s1t[:, 0:1],
                                            s1t[:, 1:2], AluOp.add)
                    stats.append(stat)
                    # block-scatter E/O rows into the xg strip tiles
                    for (ft, t, src0, dstE, dstO, nr) in SCATTER:
                        if ft != g:
                            continue
                        nc.gpsimd.dma_start(
                            xg[s][t][dstE:dstE + nr, :],
                            E[src0:src0 + nr, :])
                        nc.gpsimd.dma_start(
                            xg[s][t][dstO:dstO + nr, :],
                            O[src0:src0 + nr, :])
                return stats

            def weights(s, stats):
                """Per-band moments -> scale vector -> scaled weight tables."""
                mom = psp.tile([1, 2 * NB], F32, tag="main", name=f"mom_{s}")
                for g in range(len(FT)):
                    # partial sums -> mom[0, b0:b1] (Sx), [NB+b0:NB+b1] (Sxx)
                    b0, b1 = FT_BANDS[g]
                    nc.tensor.matmul(mom[0:1, b0:b1], lhsT=stats[g][:, 0:1],
                                     rhs=ind_sb[g][:, b0:b1],
                                     start=True, stop=True)
                    nc.tensor.matmul(mom[0:1, NB + b0:NB + b1],
                                     lhsT=stats[g][:, 1:2],
                                     rhs=ind_sb[g][:, b0:b1],
                                     start=True, stop=True)
                # moments -> s, -mu*s (everything on partition 0, free axis)
                m2 = smp.tile([1, 2 * NB], F32, tag="m2")
                nc.vector.tensor_tensor(m2[:], mom[:], invct_sb[:],
                                        AluOp.mult)   # [mu | ex2]
                mu = m2[:, 0:NB]
                ex2 = m2[:, NB:2 * NB]
                var = smp.tile([1, NB], F32, tag="var")
                nc.vector.tensor_tensor(var[:], mu, mu, AluOp.mult)  # mu^2
                nc.vector.tensor_tensor(var[:], ex2, var[:],
                                        AluOp.subtract)   # ex2 - mu^2
                sd = smp.tile([1, NB], F32, tag="sd")
                nc.scalar.activation(sd[:], var[:], ActFn.Sqrt,
                                     bias=epsc[:])
                vrow = smp.tile([1, 64], F32, tag="vrow")
                nc.vector.reciprocal(vrow[:, 0:NB], sd[:])         # s
                tmp = smp.tile([1, NB], F32, tag="tmp")
                nc.vector.tensor_tensor(tmp[:], mu, vrow[:, 0:NB],
                                        AluOp.mult)       # mu*s
                nc.vector.tensor_scalar(vrow[:, NB:2 * NB], tmp[:], -1.0, None,
                                        AluOp.mult)       # -mu*s
                nc.vector.memset(vrow[:, 62:63], 1.0)

                v63p = psp.tile([63, 1], F32, tag="main", name=f"v63p_{s}")
                nc.tensor.transpose(v63p[:], vrow[:, 0:63], ident[:])
                v63 = smp.tile([63, 1], F32, tag="v63")
                nc.vector.tensor_copy(v63[:], v63p[:])

                cvp = psp.tile([128, N_XT], F32, tag="main", name=f"cvp_{s}")
                for t in range(N_XT):
                    nc.tensor.matmul(cvp[:, t:t + 1],
                                     lhsT=msel_sb[:, t * 128:(t + 1) * 128],
                                     rhs=v63[:], start=True, stop=True)
                csb = smp.tile([128, N_XT], F32, tag="csb", name=f"csb_{s}")
                nc.vector.tensor_copy(csb[:], cvp[:])
                for t in range(N_XT):
                    c0, c1 = WT_OFF[t], WT_OFF[t] + WT_COLS[t]
                    nc.scalar.activation(wt[s][t][:], p1_sb[:, c0:c1],
                                         ActFn.Copy,
                                         scale=csb[:, t:t + 1])

            def chunk(s, t0, M):
                ob = outp.tile([128, CH * NB], F32, tag="ob", name="ob")
                # ob free index = o*31 + i  (the DRAM layout)
                ob_v = ob[0:M].rearrange("p (o i) -> p o i", o=CH, i=NB)
                for pi in range(3):
                    pt = psp.tile([128, 2048], F32, tag="main",
                                  name=f"ps{pi}")
                    for g, (blo, bhi) in enumerate(GROUP_BANDS):
                        if GROUP_PSUM[g][0] != pi:
                            continue
                        col = GROUP_PSUM[g][1]
                        t = TILE_OF_GROUP[g]
                        nb_g = bhi - blo
                        n = nb_g * CH
                        rend = REND[g]
                        gw0 = GWOFF[g]
                        if not skip_mm:
                            nc.tensor.matmul(
                                pt[0:M, col:col + n],
                                lhsT=xg[s][t][0:rend, t0:t0 + M],
                                rhs=wt[s][t][0:rend, gw0:gw0 + n],
                                start=True, stop=True)
                    # drains for this psum tensor (split across engines)
                    if skip_drain:
                        continue
                    for (dpi, kind, col, nb_g, blo, eng) in DRAINS:
                        if dpi != pi:
                            continue
                        if kind == "pair":
                            gi = col // 512
                            src = pt[0:M].rearrange(
                                "p (g r) -> p g r", g=4, r=512)[
                                :, gi:gi + 2, 0:nb_g * CH].rearrange(
                                "p g (o i) -> p o g i", o=CH, i=nb_g)
                            dst = ob_v[:, :, blo:blo + 2 * nb_g].rearrange(
                                "p o (g i) -> p o g i", g=2, i=nb_g)
                        else:
                            src = pt[0:M, col:col + nb_g * CH].rearrange(
                                "p (o i) -> p o i", o=CH, i=nb_g)
                            dst = ob_v[:, :, blo:blo + nb_g]
                        if eng == "act":
                            nc.scalar.copy(dst, src)
                        else:
                            nc.vector.tensor_copy(dst, src)
                if not skip_out:
                    nc.sync.dma_start(
                        z_d[s, t0:t0 + M].rearrange("p a b -> p (a b)"),
                        ob[0:M, :])

            for _rep in range(repeat):
                # interleaved emission: every engine queue is ordered by
                # expected data-ready time so in-order queues never block
                # early-ready work behind late-ready work.  DMA priority on
                # the sync queue: A(s0) -> p1 -> A(s1) -> out stream.
                A0 = loads(0)
                if _rep == 0:
                    nc.sync.dma_start(p1_sb[:], p1_d[:])
                A1 = loads(1)
                st0 = stats_pre(0, A0)
                weights(0, st0)
                chunk(0, *CHUNKS[0])
                st1 = stats_pre(1, A1)
                weights(1, st1)
                for (t0, M) in CHUNKS[1:n_chunks]:
                    chunk(0, t0, M)
                for (t0, M) in CHUNKS[:n_chunks]:
                    chunk(1, t0, M)

    _NC_CACHE[key] = nc
    return nc


# ----------------------------------------------------------------------------
# Public entry point
# ----------------------------------------------------------------------------
def kernel(x, gn_w, gn_b, fc_w, fc_b):
    x = np.asarray(x, np.float32)
    gn_w = np.asarray(gn_w, np.float32)
    gn_b = np.asarray(gn_b, np.float32)
    fc_w = np.asarray(fc_w, np.float32)
    fc_b = np.asarray(fc_b, np.float32)

    p1, msel, ind, invct2 = _build_const_tables(gn_w, gn_b, fc_w, fc_b)
    np_dt = np.float16 if MM_DT == "f16" else np.float32
    ones16 = np.ones((14, B_LOC * N_XT * T), np_dt)
    p1 = p1.astype(np_dt)
    x = x.astype(np_dt)
    nc = build_bass()
    if not getattr(nc, "_waits_spilled", False):
        _spill_waits(nc)
        nc._waits_spilled = True

    in_maps = []
    for k in range(N_CORES):
        in_maps.append({
            "x": np.ascontiguousarray(x[k * B_LOC:(k + 1) * B_LOC]),
            "p1": p1, "msel": msel, "ind": ind,
            "invct2": invct2, "ones16": ones16,
        })
    res = run_bass_kernel_spmd(nc, in_maps, core_ids=list(range(N_CORES)))
    z = np.concatenate([r["z"] for r in res.results], axis=0)
    return z


# revision 19
# speedup vs baseline: 1.2025x; 1.0160x over previous
"""BandSplit kernel for Trainium2 (8 NeuronCores, SPMD data-parallel over batch).

Reference computation (per band i, band width b, c=2b):
    xb[b,t,c]   = x[b, f0:f0+b, t, :] transposed/reshaped     (B, T, c)
    GroupNorm(1, c) over (T, c) per sample, affine gn_w/gn_b
    Linear(c -> 128) with fc_w/fc_b
    out stacked over 31 bands -> [B, T, 128, 31]

Key algebra: the whole band op is affine in x per sample:
    z[t,o] = s * sum_c x[t,c] * (gn_w[c]*fc_w[o,c])
             + (beta[o] + (-mu*s) * g[o])
  with s = rsqrt(var+eps), beta = fc_b + fc_w@gn_b, g = fc_w@gn_w.
The two bias terms enter the contraction through constant-1 activation
rows: one shared all-ones row carries every band's beta column block, and
one all-ones "g row" per band carries g scaled by (-mu*s).

Per-tile row layout (v2): [ones | g rows (1/band) | E rows | O rows];
everything outside the E/O blocks is constant 1.0, so the activation
tiles are memset once and only the E/O blocks are re-scattered per
sample (2 block DMAs per (xg tile, f tile) overlap = 14 per sample).

Weight columns within a matmul group are ordered (o, band) so the psum
drain writes runs of nb_g contiguous output words — the drain is the
only engine work on the output path and is split across the scalar,
vector, and gpsimd engines.
"""

import os
import numpy as np

import concourse.bass as bass
import concourse.tile as tile
import concourse.mybir as mybir
from concourse.bass_utils import run_bass_kernel_spmd

# ----------------------------------------------------------------------------
# Problem constants (hardcoded; kernel.py must be self-contained)
# ----------------------------------------------------------------------------
BANDS = [2, 3, 3, 3, 3, 3, 3, 3, 3, 3, 3, 8, 8, 8, 8, 8, 8, 8, 8, 8, 8, 8, 8,
         16, 16, 16, 16, 16, 16, 16, 17]
NB = len(BANDS)           # 31
CH = 128                  # output channels per band
EPS = 1e-5
B_FULL, F, T = 16, 257, 1000
N_CORES = 8
B_LOC = B_FULL // N_CORES  # 2 samples per core

# matmul input dtype: "f16" (1 cyc/col) or "f32" (4 cyc/col)
MM_DT = os.environ.get("BS_MM_DT", "f16")

# t-chunks of the main loop
CHUNKS = [(t0, min(128, T - t0)) for t0 in range(0, T, 128)]

# f-tiles of the raw input (aligned with band boundaries)
FT = [(0, 128), (128, 112), (240, 17)]
FT_BANDS = [(0, 23), (23, 30), (30, 31)]

# activation ("xg") tiles: bands packed so each tile stays <= 128 rows
TILE_BANDS = [(0, 13), (13, 20), (20, 25), (25, 28), (28, 31)]
N_XT = 5

# groups of <=4 bands per matmul (n = 128*nb <= 512 fits one psum bank)
GROUP_BANDS = [(0, 4), (4, 8), (8, 11), (11, 13),
               (13, 17), (17, 20),
               (20, 23), (23, 25),
               (25, 28),
               (28, 31)]
TILE_OF_GROUP = [0, 0, 0, 0, 1, 1, 2, 2, 3, 4]
# (psum_idx, col): psum tensor and column offset of each group's output.
# Adjacent same-width groups share a tensor at bank stride 512 so their
# drains merge into one instruction: {g0,g1} {g5,g6} {g8,g9}.
GROUP_PSUM = [(0, 0), (0, 512), (0, 1024), (1, 1024),
              (0, 1536), (1, 0), (1, 512), (1, 1536),
              (2, 0), (2, 512)]
# merged drain schedule: (pi, kind, col0, nb, blo, engine); kind "pair"
# drains two groups at bank stride 512 covering 2*nb adjacent bands
DRAINS = [(0, "pair", 0, 4, 0, "act"),      # g0+g1  bands 0-7
          (0, "one", 1024, 3, 8, "dve"),    # g2     bands 8-10
          (0, "one", 1536, 4, 13, "dve"),   # g4     bands 13-16
          (1, "pair", 0, 3, 17, "dve"),     # g5+g6  bands 17-22
          (1, "one", 1024, 2, 11, "act"),   # g3     bands 11-12
          (1, "one", 1536, 2, 23, "dve"),   # g7     bands 23-24
          (2, "pair", 0, 3, 25, "act")]     # g8+g9  bands 25-30


def _tile_geom():
    """Per-tile row geometry: (lo, hi, nb, SB, EST, OST, R)."""
    geom = []
    for (lo, hi) in TILE_BANDS:
        nb = hi - lo
        sb = sum(BANDS[lo:hi])
        est = 1 + nb
        ost = est + sb
        geom.append((lo, hi, nb, sb, est, ost, ost + sb))
    return geom

GEOM = _tile_geom()
TILE_ROWS = [g[6] for g in GEOM]

# wt/p1 column layout: per tile, groups concatenated; within a group the
# column order is (o, band_in_group)
WT_COLS = [g[2] * CH for g in GEOM]                       # nb_t * 128
WT_OFF = [sum(WT_COLS[:t]) for t in range(N_XT)]
TOT_COLS = sum(WT_COLS)                                   # 3968
GWOFF = []                                                # group -> local col
_acc = {}
for _g, (_blo, _bhi) in enumerate(GROUP_BANDS):
    _t = TILE_OF_GROUP[_g]
    GWOFF.append(_acc.get(_t, 0))
    _acc[_t] = GWOFF[-1] + (_bhi - _blo) * CH

# rend per group: rows [0, rend) of the tile participate in the matmul
REND = []
for _g, (_blo, _bhi) in enumerate(GROUP_BANDS):
    _t = TILE_OF_GROUP[_g]
    lo, hi, nb, sb, est, ost, rr = GEOM[_t]
    REND.append(ost + sum(BANDS[lo:_bhi]))

# E/O scatter blocks: (ft, t, src_row0, dst_E, dst_O, nrows)
SCATTER = []
for _ft, (_b0, _b1) in enumerate(FT_BANDS):
    for _t, (_lo, _hi) in enumerate(TILE_BANDS):
        ov_lo, ov_hi = max(_b0, _lo), min(_b1, _hi)
        if ov_lo >= ov_hi:
            continue
        src0 = sum(BANDS[_b0:ov_lo])
        nrows = sum(BANDS[ov_lo:ov_hi])
        lo, hi, nb, sb, est, ost, rr = GEOM[_t]
        off = sum(BANDS[_lo:ov_lo])
        SCATTER.append((_ft, _t, src0, est + off, ost + off, nrows))


def _build_const_tables(gn_w, gn_b, fc_w, fc_b):
    """Host-side packing of the (tiny) parameters into matmul-ready tables."""
    p1 = np.zeros((128, TOT_COLS), np.float32)
    msel = np.zeros((63, N_XT * 128), np.float32)
    for g, (blo, bhi) in enumerate(GROUP_BANDS):
        t = TILE_OF_GROUP[g]
        lo, hi, nb_t, sb, est, ost, rr = GEOM[t]
        nb_g = bhi - blo
        base = WT_OFF[t] + GWOFF[g]
        for j, i in enumerate(range(blo, bhi)):
            b = BANDS[i]
            c = 2 * b
            w = fc_w[i, :, :c].astype(np.float64)          # [128, c]
            beta = fc_b[i] + w @ gn_b[i, :c]               # [128]
            gv = w @ gn_w[i, :c]                           # [128]
            w2 = (w * gn_w[i, :c][None, :]).T              # [c, 128]
            cols = base + np.arange(CH) * nb_g + j
            p1[0, cols] = beta
            p1[1 + (i - lo), cols] = gv
            cumb = sum(BANDS[lo:i])
            for k in range(b):
                p1[est + cumb + k, cols] = w2[2 * k]       # E row
                p1[ost + cumb + k, cols] = w2[2 * k + 1]   # O row
    # msel: [63, N_XT*128]; csb_col(t) = msel[:, t*128:(t+1)*128]^T @ vec63
    # vec63 = [s_0..s_30, (-mu*s)_0..30, 1.0]
    for t, (lo, hi, nb_t, sb, est, ost, rr) in enumerate(GEOM):
        col = t * 128
        msel[62, col + 0] = 1.0                            # ones row: C=1
        for i in range(lo, hi):
            msel[31 + i, col + 1 + (i - lo)] = 1.0         # g row: C=-mu*s
            cumb = sum(BANDS[lo:i])
            b = BANDS[i]
            msel[i, col + est + cumb: col + est + cumb + b] = 1.0
            msel[i, col + ost + cumb: col + ost + cumb + b] = 1.0

    # Ind: [257, 31] band indicator over f rows
    ind = np.zeros((F, NB), np.float32)
    f0 = 0
    for i, b in enumerate(BANDS):
        ind[f0:f0 + b, i] = 1.0
        f0 += b

    # invCT2: [1, 62] = 1 / (c_i * T), duplicated for the Sx and Sxx halves
    invct = np.array([1.0 / (2 * b * T) for b in BANDS], np.float32)
    invct2 = np.concatenate([invct, invct])[None, :]
    return p1, msel, ind, invct2


# ----------------------------------------------------------------------------
# Bass kernel
# ----------------------------------------------------------------------------
_NC_CACHE = {}


def _spill_waits(nc):
    """Split multi-wait instructions into NoOp(wait) + instruction.

    The walrus build in this container enforces the HW wait capacity
    (1 sync wait per instruction, 2 for EventSemaphore); Tile emits more.
    Engine queues are in-order, so hoisting extra waits into preceding
    NoOps on the same queue preserves semantics.
    """
    n = 0
    for fn in nc.m.functions:
        for bb in fn.blocks:
            out = []
            changed = False
            for inst in bb.instructions:
                si = getattr(inst, "sync_info", None)
                cap = 2 if isinstance(inst, mybir.InstEventSemaphore) else 1
                if si is not None and si.on_wait and len(si.on_wait) > cap:
                    waits = list(si.on_wait)
                    extra, keep = waits[:-cap], waits[-cap:]
                    for w in extra:
                        nop = mybir.InstNoOp(name=f"{inst.name}_w{n}",
                                             ins=[], outs=[])
                        nop.engine = inst.engine
                        nop.sync_info = mybir.SyncInfo(on_wait=[w],
                                                       on_update=[])
                        out.append(nop)
                        n += 1
                    si.on_wait = keep
                    changed = True
                out.append(inst)
            if changed:
                bb.instructions = out
    return n


def _mm_dt():
    return mybir.dt.float32 if MM_DT == "f32" else mybir.dt.float16


def build_bass():
    repeat = int(os.environ.get("BS_REPEAT", "1"))
    key = (MM_DT, repeat)
    if key in _NC_CACHE:
        return _NC_CACHE[key]
    DT = _mm_dt()
    F32 = mybir.dt.float32

    nc = bass.Bass("TRN2", target_bir_lowering=False, debug=False,
                   num_devices=N_CORES)

    x_d = nc.dram_tensor("x", [B_LOC, F, T, 2], DT, kind="ExternalInput").ap()
    p1_d = nc.dram_tensor("p1", [128, TOT_COLS], DT, kind="ExternalInput").ap()
    msel_d = nc.dram_tensor("msel", [63, N_XT * 128], F32, kind="ExternalInput").ap()
    ind_d = nc.dram_tensor("ind", [F, NB], F32, kind="ExternalInput").ap()
    invct_d = nc.dram_tensor("invct2", [1, 2 * NB], F32, kind="ExternalInput").ap()
    ones_d = nc.dram_tensor("ones16", [14, B_LOC * N_XT * T], DT,
                            kind="ExternalInput").ap()
    z_d = nc.dram_tensor("z", [B_LOC, T, CH, NB], F32, kind="ExternalOutput").ap()

    AluOp = mybir.AluOpType
    ActFn = mybir.ActivationFunctionType

    with tile.TileContext(nc) as tc:
        with (
            tc.tile_pool(name="const", bufs=1) as constp,
            tc.tile_pool(name="a", bufs=6) as ap_,
            tc.tile_pool(name="eo", bufs=6) as eop,
            tc.tile_pool(name="xg", bufs=1) as xgp,
            tc.tile_pool(name="wt", bufs=1) as wtp,
            tc.tile_pool(name="small", bufs=8) as smp,
            tc.tile_pool(name="out", bufs=3) as outp,
            tc.tile_pool(name="sq", bufs=1) as sqp,
            tc.tile_pool(name="psum", bufs=2, space="PSUM") as psp,
        ):
            # ---------------- constants to SBUF ----------------
            p1_sb = constp.tile([128, TOT_COLS], DT, tag="p1",
                                name="p1c")
            msel_sb = constp.tile([63, N_XT * 128], F32, tag="msel")
            nc.sync.dma_start(msel_sb[:], msel_d[:])
            ind_sb = []
            for g, (f0, P) in enumerate(FT):
                it = constp.tile([P, NB], F32, tag=f"ind_{g}", name=f"indc_{g}")
                nc.sync.dma_start(it[:], ind_d[f0:f0 + P, :])
                ind_sb.append(it)
            invct_sb = constp.tile([1, 2 * NB], F32, tag="invct")
            nc.sync.dma_start(invct_sb[:], invct_d[:])
            ident = constp.tile([1, 1], F32, tag="ident")
            nc.vector.memset(ident[:], 1.0)
            zcol = constp.tile([128, 1], F32, tag="zcol")
            nc.vector.memset(zcol[:], 0.0)
            epsc = constp.tile([1, 1], F32, tag="epsc")
            nc.vector.memset(epsc[:], EPS)
            warm = constp.tile([1, 1], F32, tag="warm")
            nc.scalar.activation(warm[:], epsc[:], ActFn.Square,
                                 bias=zcol[0:1, :])

            # persistent per-(sample,tile) tensors — separate tensors keep
            # the tile framework's dependency tracking fine-grained.  The
            # constant ones+g rows load once from DRAM; E/O blocks are
            # re-scattered per sample.
            xg = [[xgp.tile([128, T], DT, tag=f"xg_{s}_{t}", name=f"xg_{s}_{t}")
                   for t in range(N_XT)] for s in range(B_LOC)]
            wt = [[wtp.tile([128, WT_COLS[t]], DT, tag=f"wt_{s}_{t}",
                            name=f"wt_{s}_{t}")
                   for t in range(N_XT)] for s in range(B_LOC)]
            for s in range(B_LOC):
                for t in range(N_XT):
                    est = GEOM[t][4]
                    nc.gpsimd.dma_start(xg[s][t][0:est, :],
                                        ones_d[0:est, 0:T])

            # ---------------- body (repeatable for benchmarking) ------
            n_chunks = int(os.environ.get("BS_NCHUNKS", "8"))
            skip_mm = os.environ.get("BS_SKIP_MM") == "1"
            skip_drain = os.environ.get("BS_SKIP_DRAIN") == "1"
            skip_out = os.environ.get("BS_SKIP_OUT") == "1"

            def loads(s, queue=None):
                """Input loads for sample s (default: sync DMA queue)."""
                q = queue if queue is not None else nc.sync
                As = []
                for g, (f0, P) in enumerate(FT):
                    A = ap_.tile([P, 2000], DT, tag="a", name=f"A_{s}_{g}")
                    q.dma_start(
                        A[:], x_d[s, f0:f0 + P].rearrange("p a b -> p (a b)"))
                    As.append(A)
                return As

            def stats_ft(s, As, g):
                """square+dei+row sums + E/O scatter for one f-tile."""
                if True:
                    f0, P = FT[g]
                    A = As[g]
                    stat = smp.tile([P, 2], F32, tag="stat",
                                    name=f"stat_{s}_{g}")
                    s1t = smp.tile([P, 2], F32, tag="s1t", name=f"s1t_{s}_{g}")
                    # fused square + per-row sum on the scalar engine
                    Asq = sqp.tile([P, 2000], DT, tag="sq", name="Asq")
                    nc.scalar.activation(Asq[:], A[:], ActFn.Square,
                                         bias=zcol[0:P, :],
                                         accum_out=stat[:, 1:2])
                    # de-interleave (and cast); accumulate sums per f-row
                    Av = A[:].rearrange("p (t r) -> p r t", r=2)
                    E = eop.tile([P, T], DT, tag="eo", name=f"E_{s}_{g}")
                    O = eop.tile([P, T], DT, tag="eo", name=f"O_{s}_{g}")
                    nc.vector.tensor_scalar(E[:], Av[:, 0, :], 1.0, None,
                                            AluOp.mult, AluOp.add,
                                            accum_out=s1t[:, 0:1])
                    nc.vector.tensor_scalar(O[:], Av[:, 1, :], 1.0, None,
                                            AluOp.mult, AluOp.add,
                                            accum_out=s1t[:, 1:2])
                    nc.vector.tensor_tensor(stat[:, 0:1], s1t[:, 0:1],
                                            s1t[:, 1:2], AluOp.add)
                    # block-scatter E/O rows into the xg strip tiles
                    for (ft, t, src0, dstE, dstO, nr) in SCATTER:
                        if ft != g:
                            continue
                        nc.gpsimd.dma_start(
                            xg[s][t][dstE:dstE + nr, :],
                            E[src0:src0 + nr, :])
                        nc.gpsimd.dma_start(
                            xg[s][t][dstO:dstO + nr, :],
                            O[src0:src0 + nr, :])
                return stat

            def weights(s, stats):
                """Per-band moments -> scale vector -> scaled weight tables."""
                mom = psp.tile([1, 2 * NB], F32, tag="main", name=f"mom_{s}")
                for g in range(len(FT)):
                    # partial sums -> mom[0, b0:b1] (Sx), [NB+b0:NB+b1] (Sxx)
                    b0, b1 = FT_BANDS[g]
                    nc.tensor.matmul(mom[0:1, b0:b1], lhsT=stats[g][:, 0:1],
                                     rhs=ind_sb[g][:, b0:b1],
                                     start=True, stop=True)
                    nc.tensor.matmul(mom[0:1, NB + b0:NB + b1],
                                     lhsT=stats[g][:, 1:2],
                                     rhs=ind_sb[g][:, b0:b1],
                                     start=True, stop=True)
                # moments -> s, -mu*s (everything on partition 0, free axis)
                m2 = smp.tile([1, 2 * NB], F32, tag="m2")
                nc.vector.tensor_tensor(m2[:], mom[:], invct_sb[:],
                                        AluOp.mult)   # [mu | ex2]
                mu = m2[:, 0:NB]
                ex2 = m2[:, NB:2 * NB]
                var = smp.tile([1, NB], F32, tag="var")
                nc.vector.tensor_tensor(var[:], mu, mu, AluOp.mult)  # mu^2
                nc.vector.tensor_tensor(var[:], ex2, var[:],
                                        AluOp.subtract)   # ex2 - mu^2
                sd = smp.tile([1, NB], F32, tag="sd")
                nc.scalar.activation(sd[:], var[:], ActFn.Sqrt,
                                     bias=epsc[:])
                vrow = smp.tile([1, 64], F32, tag="vrow")
                nc.vector.reciprocal(vrow[:, 0:NB], sd[:])         # s
                tmp = smp.tile([1, NB], F32, tag="tmp")
                nc.vector.tensor_tensor(tmp[:], mu, vrow[:, 0:NB],
                                        AluOp.mult)       # mu*s
                nc.vector.tensor_scalar(vrow[:, NB:2 * NB], tmp[:], -1.0, None,
                                        AluOp.mult)       # -mu*s
                nc.vector.memset(vrow[:, 62:63], 1.0)

                v63p = psp.tile([63, 1], F32, tag="main", name=f"v63p_{s}")
                nc.tensor.transpose(v63p[:], vrow[:, 0:63], ident[:])
                v63 = smp.tile([63, 1], F32, tag="v63")
                nc.vector.tensor_copy(v63[:], v63p[:])

                cvp = psp.tile([128, N_XT], F32, tag="main", name=f"cvp_{s}")
                for t in range(N_XT):
                    nc.tensor.matmul(cvp[:, t:t + 1],
                                     lhsT=msel_sb[:, t * 128:(t + 1) * 128],
                                     rhs=v63[:], start=True, stop=True)
                csb = smp.tile([128, N_XT], F32, tag="csb", name=f"csb_{s}")
                nc.vector.tensor_copy(csb[:], cvp[:])
                for t in range(N_XT):
                    c0, c1 = WT_OFF[t], WT_OFF[t] + WT_COLS[t]
                    if t < 2:
                        nc.scalar.activation(wt[s][t][:], p1_sb[:, c0:c1],
                                             ActFn.Copy,
                                             scale=csb[:, t:t + 1])
                    else:
                        nc.vector.tensor_scalar(wt[s][t][:], p1_sb[:, c0:c1],
                                                csb[:, t:t + 1], None,
                                                AluOp.mult)

            def chunk(s, t0, M):
                ob = outp.tile([128, CH * NB], F32, tag="ob", name="ob")
                # ob free index = o*31 + i  (the DRAM layout)
                ob_v = ob[0:M].rearrange("p (o i) -> p o i", o=CH, i=NB)
                for pi in range(3):
                    pt = psp.tile([128, 2048], F32, tag="main",
                                  name=f"ps{pi}")
                    for g, (blo, bhi) in enumerate(GROUP_BANDS):
                        if GROUP_PSUM[g][0] != pi:
                            continue
                        col = GROUP_PSUM[g][1]
                        t = TILE_OF_GROUP[g]
                        nb_g = bhi - blo
                        n = nb_g * CH
                        rend = REND[g]
                        gw0 = GWOFF[g]
                        if not skip_mm:
                            nc.tensor.matmul(
                                pt[0:M, col:col + n],
                                lhsT=xg[s][t][0:rend, t0:t0 + M],
                                rhs=wt[s][t][0:rend, gw0:gw0 + n],
                                start=True, stop=True)
                    # drains for this psum tensor (split across engines)
                    if skip_drain:
                        continue
                    for (dpi, kind, col, nb_g, blo, eng) in DRAINS:
                        if dpi != pi:
                            continue
                        if kind == "pair":
                            gi = col // 512
                            src = pt[0:M].rearrange(
                                "p (g r) -> p g r", g=4, r=512)[
                                :, gi:gi + 2, 0:nb_g * CH].rearrange(
                                "p g (o i) -> p o g i", o=CH, i=nb_g)
                            dst = ob_v[:, :, blo:blo + 2 * nb_g].rearrange(
                                "p o (g i) -> p o g i", g=2, i=nb_g)
                        else:
                            src = pt[0:M, col:col + nb_g * CH].rearrange(
                                "p (o i) -> p o i", o=CH, i=nb_g)
                            dst = ob_v[:, :, blo:blo + nb_g]
                        if eng == "act":
                            nc.scalar.copy(dst, src)
                        else:
                            nc.vector.tensor_copy(dst, src)
                if not skip_out:
                    nc.sync.dma_start(
                        z_d[s, t0:t0 + M].rearrange("p a b -> p (a b)"),
                        ob[0:M, :])

            for _rep in range(repeat):
                # interleaved emission: every engine queue is ordered by
                # expected data-ready time so in-order queues never block
                # early-ready work behind late-ready work.  DMA priority on
                # the sync queue: A(s0) -> p1 -> A(s1) -> out stream.
                A0 = loads(0)
                if _rep == 0:
                    nc.sync.dma_start(p1_sb[:], p1_d[:])
                A1 = loads(1)
                st0 = [stats_ft(0, A0, g) for g in range(3)]
                weights(0, st0)
                chunk(0, *CHUNKS[0])
                st1 = [stats_ft(1, A1, 0)]
                chunk(0, *CHUNKS[1])
                st1.append(stats_ft(1, A1, 1))
                chunk(0, *CHUNKS[2])
                st1.append(stats_ft(1, A1, 2))
                chunk(0, *CHUNKS[3])
                weights(1, st1)
                for (t0, M) in CHUNKS[4:n_chunks]:
                    chunk(0, t0, M)
                for (t0, M) in CHUNKS[:n_chunks]:
                    chunk(1, t0, M)

    _NC_CACHE[key] = nc
    return nc


# ----------------------------------------------------------------------------
# Public entry point
# ----------------------------------------------------------------------------
def kernel(x, gn_w, gn_b, fc_w, fc_b):
    x = np.asarray(x, np.float32)
    gn_w = np.asarray(gn_w, np.float32)
    gn_b = np.asarray(gn_b, np.float32)
    fc_w = np.asarray(fc_w, np.float32)
    fc_b = np.asarray(fc_b, np.float32)

    p1, msel, ind, invct2 = _build_const_tables(gn_w, gn_b, fc_w, fc_b)
    np_dt = np.float16 if MM_DT == "f16" else np.float32
    ones16 = np.ones((14, B_LOC * N_XT * T), np_dt)
    p1 = p1.astype(np_dt)
    x = x.astype(np_dt)
    nc = build_bass()
    if not getattr(nc, "_waits_spilled", False):
        _spill_waits(nc)
        nc._waits_spilled = True

    in_maps = []
    for k in range(N_CORES):
        in_maps.append({
            "x": np.ascontiguousarray(x[k * B_LOC:(k + 1) * B_LOC]),
            "p1": p1, "msel": msel, "ind": ind,
            "invct2": invct2, "ones16": ones16,
        })
    res = run_bass_kernel_spmd(nc, in_maps, core_ids=list(range(N_CORES)))
    z = np.concatenate([r["z"] for r in res.results], axis=0)
    return z


# revision 20
# speedup vs baseline: 1.2298x; 1.0227x over previous
"""BandSplit kernel for Trainium2 (8 NeuronCores, SPMD data-parallel over batch).

Reference computation (per band i, band width b, c=2b):
    xb[b,t,c]   = x[b, f0:f0+b, t, :] transposed/reshaped     (B, T, c)
    GroupNorm(1, c) over (T, c) per sample, affine gn_w/gn_b
    Linear(c -> 128) with fc_w/fc_b
    out stacked over 31 bands -> [B, T, 128, 31]

Key algebra: the whole band op is affine in x per sample:
    z[t,o] = s * sum_c x[t,c] * (gn_w[c]*fc_w[o,c])
             + (beta[o] + (-mu*s) * g[o])
  with s = rsqrt(var+eps), beta = fc_b + fc_w@gn_b, g = fc_w@gn_w.
The two bias terms enter the contraction through constant-1 activation
rows: one shared all-ones row carries every band's beta column block, and
one all-ones "g row" per band carries g scaled by (-mu*s).

Per-tile row layout (v2): [ones | g rows (1/band) | E rows | O rows];
everything outside the E/O blocks is constant 1.0, so the activation
tiles are memset once and only the E/O blocks are re-scattered per
sample (2 block DMAs per (xg tile, f tile) overlap = 14 per sample).

Weight columns within a matmul group are ordered (o, band) so the psum
drain writes runs of nb_g contiguous output words — the drain is the
only engine work on the output path and is split across the scalar,
vector, and gpsimd engines.
"""

import os
import numpy as np

import concourse.bass as bass
import concourse.tile as tile
import concourse.mybir as mybir
from concourse.bass_utils import run_bass_kernel_spmd

# ----------------------------------------------------------------------------
# Problem constants (hardcoded; kernel.py must be self-contained)
# ----------------------------------------------------------------------------
BANDS = [2, 3, 3, 3, 3, 3, 3, 3, 3, 3, 3, 8, 8, 8, 8, 8, 8, 8, 8, 8, 8, 8, 8,
         16, 16, 16, 16, 16, 16, 16, 17]
NB = len(BANDS)           # 31
CH = 128                  # output channels per band
EPS = 1e-5
B_FULL, F, T = 16, 257, 1000
N_CORES = 8
B_LOC = B_FULL // N_CORES  # 2 samples per core

# matmul input dtype: "f16" (1 cyc/col) or "f32" (4 cyc/col)
MM_DT = os.environ.get("BS_MM_DT", "f16")

# t-chunks of the main loop
CHUNKS = [(t0, min(128, T - t0)) for t0 in range(0, T, 128)]

# f-tiles of the raw input (aligned with band boundaries)
FT = [(0, 128), (128, 112), (240, 17)]
FT_BANDS = [(0, 23), (23, 30), (30, 31)]

# activation ("xg") tiles: bands packed so each tile stays <= 128 rows
TILE_BANDS = [(0, 13), (13, 20), (20, 25), (25, 28), (28, 31)]
N_XT = 5

# groups of <=4 bands per matmul (n = 128*nb <= 512 fits one psum bank)
GROUP_BANDS = [(0, 4), (4, 8), (8, 11), (11, 13),
               (13, 17), (17, 20),
               (20, 23), (23, 25),
               (25, 28),
               (28, 31)]
TILE_OF_GROUP = [0, 0, 0, 0, 1, 1, 2, 2, 3, 4]
# (psum_idx, col): psum tensor and column offset of each group's output.
# Adjacent same-width groups share a tensor at bank stride 512 so their
# drains merge into one instruction: {g0,g1} {g5,g6} {g8,g9}.
GROUP_PSUM = [(0, 0), (0, 512), (0, 1024), (1, 1024),
              (0, 1536), (1, 0), (1, 512), (1, 1536),
              (2, 0), (2, 512)]
# merged drain schedule: (pi, kind, col0, nb, blo, engine); kind "pair"
# drains two groups at bank stride 512 covering 2*nb adjacent bands
DRAINS = [(0, "pair", 0, 4, 0, "act"),      # g0+g1  bands 0-7
          (0, "one", 1024, 3, 8, "dve"),    # g2     bands 8-10
          (0, "one", 1536, 4, 13, "dve"),   # g4     bands 13-16
          (1, "pair", 0, 3, 17, "dve"),     # g5+g6  bands 17-22
          (1, "one", 1024, 2, 11, "act"),   # g3     bands 11-12
          (1, "one", 1536, 2, 23, "dve"),   # g7     bands 23-24
          (2, "pair", 0, 3, 25, "act")]     # g8+g9  bands 25-30


def _tile_geom():
    """Per-tile row geometry: (lo, hi, nb, SB, EST, OST, R)."""
    geom = []
    for (lo, hi) in TILE_BANDS:
        nb = hi - lo
        sb = sum(BANDS[lo:hi])
        est = 1 + nb
        ost = est + sb
        geom.append((lo, hi, nb, sb, est, ost, ost + sb))
    return geom

GEOM = _tile_geom()
TILE_ROWS = [g[6] for g in GEOM]

# wt/p1 column layout: per tile, groups concatenated; within a group the
# column order is (o, band_in_group)
WT_COLS = [g[2] * CH for g in GEOM]                       # nb_t * 128
WT_OFF = [sum(WT_COLS[:t]) for t in range(N_XT)]
TOT_COLS = sum(WT_COLS)                                   # 3968
GWOFF = []                                                # group -> local col
_acc = {}
for _g, (_blo, _bhi) in enumerate(GROUP_BANDS):
    _t = TILE_OF_GROUP[_g]
    GWOFF.append(_acc.get(_t, 0))
    _acc[_t] = GWOFF[-1] + (_bhi - _blo) * CH

# rend per group: rows [0, rend) of the tile participate in the matmul
REND = []
for _g, (_blo, _bhi) in enumerate(GROUP_BANDS):
    _t = TILE_OF_GROUP[_g]
    lo, hi, nb, sb, est, ost, rr = GEOM[_t]
    REND.append(ost + sum(BANDS[lo:_bhi]))

# E/O scatter blocks: (ft, t, src_row0, dst_E, dst_O, nrows)
SCATTER = []
for _ft, (_b0, _b1) in enumerate(FT_BANDS):
    for _t, (_lo, _hi) in enumerate(TILE_BANDS):
        ov_lo, ov_hi = max(_b0, _lo), min(_b1, _hi)
        if ov_lo >= ov_hi:
            continue
        src0 = sum(BANDS[_b0:ov_lo])
        nrows = sum(BANDS[ov_lo:ov_hi])
        lo, hi, nb, sb, est, ost, rr = GEOM[_t]
        off = sum(BANDS[_lo:ov_lo])
        SCATTER.append((_ft, _t, src0, est + off, ost + off, nrows))


def _build_const_tables(gn_w, gn_b, fc_w, fc_b):
    """Host-side packing of the (tiny) parameters into matmul-ready tables."""
    p1 = np.zeros((128, TOT_COLS), np.float32)
    msel = np.zeros((63, N_XT * 128), np.float32)
    for g, (blo, bhi) in enumerate(GROUP_BANDS):
        t = TILE_OF_GROUP[g]
        lo, hi, nb_t, sb, est, ost, rr = GEOM[t]
        nb_g = bhi - blo
        base = WT_OFF[t] + GWOFF[g]
        for j, i in enumerate(range(blo, bhi)):
            b = BANDS[i]
            c = 2 * b
            w = fc_w[i, :, :c].astype(np.float64)          # [128, c]
            beta = fc_b[i] + w @ gn_b[i, :c]               # [128]
            gv = w @ gn_w[i, :c]                           # [128]
            w2 = (w * gn_w[i, :c][None, :]).T              # [c, 128]
            cols = base + np.arange(CH) * nb_g + j
            p1[0, cols] = beta
            p1[1 + (i - lo), cols] = gv
            cumb = sum(BANDS[lo:i])
            for k in range(b):
                p1[est + cumb + k, cols] = w2[2 * k]       # E row
                p1[ost + cumb + k, cols] = w2[2 * k + 1]   # O row
    # msel: [63, N_XT*128]; csb_col(t) = msel[:, t*128:(t+1)*128]^T @ vec63
    # vec63 = [s_0..s_30, (-mu*s)_0..30, 1.0]
    for t, (lo, hi, nb_t, sb, est, ost, rr) in enumerate(GEOM):
        col = t * 128
        msel[62, col + 0] = 1.0                            # ones row: C=1
        for i in range(lo, hi):
            msel[31 + i, col + 1 + (i - lo)] = 1.0         # g row: C=-mu*s
            cumb = sum(BANDS[lo:i])
            b = BANDS[i]
            msel[i, col + est + cumb: col + est + cumb + b] = 1.0
            msel[i, col + ost + cumb: col + ost + cumb + b] = 1.0

    # Ind: [257, 31] band indicator over f rows
    ind = np.zeros((F, NB), np.float32)
    f0 = 0
    for i, b in enumerate(BANDS):
        ind[f0:f0 + b, i] = 1.0
        f0 += b

    # invCT2: [1, 62] = 1 / (c_i * T), duplicated for the Sx and Sxx halves
    invct = np.array([1.0 / (2 * b * T) for b in BANDS], np.float32)
    invct2 = np.concatenate([invct, invct])[None, :]
    return p1, msel, ind, invct2


# ----------------------------------------------------------------------------
# Bass kernel
# ----------------------------------------------------------------------------
_NC_CACHE = {}


def _spill_waits(nc):
    """Split multi-wait instructions into NoOp(wait) + instruction.

    The walrus build in this container enforces the HW wait capacity
    (1 sync wait per instruction, 2 for EventSemaphore); Tile emits more.
    Engine queues are in-order, so hoisting extra waits into preceding
    NoOps on the same queue preserves semantics.
    """
    n = 0
    for fn in nc.m.functions:
        for bb in fn.blocks:
            out = []
            changed = False
            for inst in bb.instructions:
                si = getattr(inst, "sync_info", None)
                cap = 2 if isinstance(inst, mybir.InstEventSemaphore) else 1
                if si is not None and si.on_wait and len(si.on_wait) > cap:
                    waits = list(si.on_wait)
                    extra, keep = waits[:-cap], waits[-cap:]
                    for w in extra:
                        nop = mybir.InstNoOp(name=f"{inst.name}_w{n}",
                                             ins=[], outs=[])
                        nop.engine = inst.engine
                        nop.sync_info = mybir.SyncInfo(on_wait=[w],
                                                       on_update=[])
                        out.append(nop)
                        n += 1
                    si.on_wait = keep
                    changed = True
                out.append(inst)
            if changed:
                bb.instructions = out
    return n


def _mm_dt():
    return mybir.dt.float32 if MM_DT == "f32" else mybir.dt.float16


def build_bass():
    repeat = int(os.environ.get("BS_REPEAT", "1"))
    key = (MM_DT, repeat)
    if key in _NC_CACHE:
        return _NC_CACHE[key]
    DT = _mm_dt()
    F32 = mybir.dt.float32

    nc = bass.Bass("TRN2", target_bir_lowering=False, debug=False,
                   num_devices=N_CORES)

    x_d = nc.dram_tensor("x", [B_LOC, F, T, 2], DT, kind="ExternalInput").ap()
    p1_d = nc.dram_tensor("p1", [128, TOT_COLS], DT, kind="ExternalInput").ap()
    msel_d = nc.dram_tensor("msel", [63, N_XT * 128], F32, kind="ExternalInput").ap()
    ind_d = nc.dram_tensor("ind", [F, NB], F32, kind="ExternalInput").ap()
    invct_d = nc.dram_tensor("invct2", [1, 2 * NB], F32, kind="ExternalInput").ap()
    ones_d = nc.dram_tensor("ones16", [14, B_LOC * N_XT * T], DT,
                            kind="ExternalInput").ap()
    z_d = nc.dram_tensor("z", [B_LOC, T, CH, NB], F32, kind="ExternalOutput").ap()

    AluOp = mybir.AluOpType
    ActFn = mybir.ActivationFunctionType

    with tile.TileContext(nc) as tc:
        with (
            tc.tile_pool(name="const", bufs=1) as constp,
            tc.tile_pool(name="a", bufs=6) as ap_,
            tc.tile_pool(name="eo", bufs=6) as eop,
            tc.tile_pool(name="xg", bufs=1) as xgp,
            tc.tile_pool(name="wt", bufs=1) as wtp,
            tc.tile_pool(name="small", bufs=8) as smp,
            tc.tile_pool(name="out", bufs=3) as outp,
            tc.tile_pool(name="sq", bufs=1) as sqp,
            tc.tile_pool(name="psum", bufs=2, space="PSUM") as psp,
        ):
            # ---------------- constants to SBUF ----------------
            p1_sb = constp.tile([128, TOT_COLS], DT, tag="p1",
                                name="p1c")
            msel_sb = constp.tile([63, N_XT * 128], F32, tag="msel")
            nc.sync.dma_start(msel_sb[:], msel_d[:])
            ind_sb = []
            for g, (f0, P) in enumerate(FT):
                it = constp.tile([P, NB], F32, tag=f"ind_{g}", name=f"indc_{g}")
                nc.sync.dma_start(it[:], ind_d[f0:f0 + P, :])
                ind_sb.append(it)
            invct_sb = constp.tile([1, 2 * NB], F32, tag="invct")
            nc.sync.dma_start(invct_sb[:], invct_d[:])
            ident = constp.tile([1, 1], F32, tag="ident")
            nc.vector.memset(ident[:], 1.0)
            zcol = constp.tile([128, 1], F32, tag="zcol")
            nc.vector.memset(zcol[:], 0.0)
            epsc = constp.tile([1, 1], F32, tag="epsc")
            nc.vector.memset(epsc[:], EPS)
            stg = constp.tile([1, 1], DT, tag="stg")
            warm = constp.tile([1, 1], F32, tag="warm")
            nc.scalar.activation(warm[:], epsc[:], ActFn.Square,
                                 bias=zcol[0:1, :])

            # persistent per-(sample,tile) tensors — separate tensors keep
            # the tile framework's dependency tracking fine-grained.  The
            # constant ones+g rows load once from DRAM; E/O blocks are
            # re-scattered per sample.
            xg = [[xgp.tile([128, T], DT, tag=f"xg_{s}_{t}", name=f"xg_{s}_{t}")
                   for t in range(N_XT)] for s in range(B_LOC)]
            wt = [[wtp.tile([128, WT_COLS[t]], DT, tag=f"wt_{s}_{t}",
                            name=f"wt_{s}_{t}")
                   for t in range(N_XT)] for s in range(B_LOC)]
            for s in range(B_LOC):
                for t in range(N_XT):
                    est = GEOM[t][4]
                    nc.gpsimd.dma_start(xg[s][t][0:est, :],
                                        ones_d[0:est, 0:T])

            # ---------------- body (repeatable for benchmarking) ------
            n_chunks = int(os.environ.get("BS_NCHUNKS", "8"))
            skip_mm = os.environ.get("BS_SKIP_MM") == "1"
            skip_drain = os.environ.get("BS_SKIP_DRAIN") == "1"
            skip_out = os.environ.get("BS_SKIP_OUT") == "1"

            def loads(s, queue=None):
                """Input loads for sample s (default: sync DMA queue)."""
                q = queue if queue is not None else nc.sync
                As = []
                for g, (f0, P) in enumerate(FT):
                    A = ap_.tile([P, 2000], DT, tag="a", name=f"A_{s}_{g}")
                    q.dma_start(
                        A[:], x_d[s, f0:f0 + P].rearrange("p a b -> p (a b)"))
                    As.append(A)
                return As

            def stats_ft(s, As, g):
                """square+dei+row sums + E/O scatter for one f-tile."""
                if True:
                    f0, P = FT[g]
                    A = As[g]
                    stat = smp.tile([P, 2], F32, tag="stat",
                                    name=f"stat_{s}_{g}")
                    s1t = smp.tile([P, 2], F32, tag="s1t", name=f"s1t_{s}_{g}")
                    # fused square + per-row sum on the scalar engine
                    Asq = sqp.tile([P, 2000], DT, tag="sq", name="Asq")
                    nc.scalar.activation(Asq[:], A[:], ActFn.Square,
                                         bias=zcol[0:P, :],
                                         accum_out=stat[:, 1:2])
                    # de-interleave (and cast); accumulate sums per f-row
                    Av = A[:].rearrange("p (t r) -> p r t", r=2)
                    E = eop.tile([P, T], DT, tag="eo", name=f"E_{s}_{g}")
                    O = eop.tile([P, T], DT, tag="eo", name=f"O_{s}_{g}")
                    nc.vector.tensor_scalar(E[:], Av[:, 0, :], 1.0, None,
                                            AluOp.mult, AluOp.add,
                                            accum_out=s1t[:, 0:1])
                    nc.vector.tensor_scalar(O[:], Av[:, 1, :], 1.0, None,
                                            AluOp.mult, AluOp.add,
                                            accum_out=s1t[:, 1:2])
                    nc.vector.tensor_tensor(stat[:, 0:1], s1t[:, 0:1],
                                            s1t[:, 1:2], AluOp.add)
                    # block-scatter E/O rows into the xg strip tiles
                    for (ft, t, src0, dstE, dstO, nr) in SCATTER:
                        if ft != g:
                            continue
                        nc.gpsimd.dma_start(
                            xg[s][t][dstE:dstE + nr, :],
                            E[src0:src0 + nr, :])
                        nc.gpsimd.dma_start(
                            xg[s][t][dstO:dstO + nr, :],
                            O[src0:src0 + nr, :])
                return stat

            def weights(s, stats):
                """Per-band moments -> scale vector -> scaled weight tables."""
                mom = psp.tile([1, 2 * NB], F32, tag="main", name=f"mom_{s}")
                for g in range(len(FT)):
                    # partial sums -> mom[0, b0:b1] (Sx), [NB+b0:NB+b1] (Sxx)
                    b0, b1 = FT_BANDS[g]
                    nc.tensor.matmul(mom[0:1, b0:b1], lhsT=stats[g][:, 0:1],
                                     rhs=ind_sb[g][:, b0:b1],
                                     start=True, stop=True)
                    nc.tensor.matmul(mom[0:1, NB + b0:NB + b1],
                                     lhsT=stats[g][:, 1:2],
                                     rhs=ind_sb[g][:, b0:b1],
                                     start=True, stop=True)
                # moments -> s, -mu*s (everything on partition 0, free axis)
                m2 = smp.tile([1, 2 * NB], F32, tag="m2")
                nc.vector.tensor_tensor(m2[:], mom[:], invct_sb[:],
                                        AluOp.mult)   # [mu | ex2]
                mu = m2[:, 0:NB]
                ex2 = m2[:, NB:2 * NB]
                var = smp.tile([1, NB], F32, tag="var")
                nc.vector.tensor_tensor(var[:], mu, mu, AluOp.mult)  # mu^2
                nc.vector.tensor_tensor(var[:], ex2, var[:],
                                        AluOp.subtract)   # ex2 - mu^2
                sd = smp.tile([1, NB], F32, tag="sd")
                nc.scalar.activation(sd[:], var[:], ActFn.Sqrt,
                                     bias=epsc[:])
                vrow = smp.tile([1, 64], F32, tag="vrow")
                nc.vector.reciprocal(vrow[:, 0:NB], sd[:])         # s
                nc.vector.scalar_tensor_tensor(vrow[:, NB:2 * NB], mu, -1.0,
                                               vrow[:, 0:NB], AluOp.mult,
                                               AluOp.mult)  # -mu*s
                nc.vector.memset(vrow[:, 62:63], 1.0)

                v63p = psp.tile([63, 1], F32, tag="main", name=f"v63p_{s}")
                nc.tensor.transpose(v63p[:], vrow[:, 0:63], ident[:])
                v63 = smp.tile([63, 1], F32, tag="v63")
                nc.vector.tensor_copy(v63[:], v63p[:])

                cvp = psp.tile([128, N_XT], F32, tag="main", name=f"cvp_{s}")
                for t in range(N_XT):
                    nc.tensor.matmul(cvp[:, t:t + 1],
                                     lhsT=msel_sb[:, t * 128:(t + 1) * 128],
                                     rhs=v63[:], start=True, stop=True)
                csb = smp.tile([128, N_XT], F32, tag="csb", name=f"csb_{s}")
                nc.vector.tensor_copy(csb[:], cvp[:])
                return csb

            def wt_build(s, csb):
                for t in range(N_XT):
                    c0, c1 = WT_OFF[t], WT_OFF[t] + WT_COLS[t]
                    if t < 2:
                        nc.scalar.activation(wt[s][t][:], p1_sb[:, c0:c1],
                                             ActFn.Copy,
                                             scale=csb[:, t:t + 1])
                    else:
                        nc.vector.tensor_scalar(wt[s][t][:], p1_sb[:, c0:c1],
                                                csb[:, t:t + 1], None,
                                                AluOp.mult)

            def chunk(s, t0, M):
                ob = outp.tile([128, CH * NB], F32, tag="ob", name="ob")
                # ob free index = o*31 + i  (the DRAM layout)
                ob_v = ob[0:M].rearrange("p (o i) -> p o i", o=CH, i=NB)
                for pi in range(3):
                    pt = psp.tile([128, 2048], F32, tag="main",
                                  name=f"ps{pi}")
                    for g, (blo, bhi) in enumerate(GROUP_BANDS):
                        if GROUP_PSUM[g][0] != pi:
                            continue
                        col = GROUP_PSUM[g][1]
                        t = TILE_OF_GROUP[g]
                        nb_g = bhi - blo
                        n = nb_g * CH
                        rend = REND[g]
                        gw0 = GWOFF[g]
                        if not skip_mm:
                            nc.tensor.matmul(
                                pt[0:M, col:col + n],
                                lhsT=xg[s][t][0:rend, t0:t0 + M],
                                rhs=wt[s][t][0:rend, gw0:gw0 + n],
                                start=True, stop=True)
                    # drains for this psum tensor (split across engines)
                    if skip_drain:
                        continue
                    for (dpi, kind, col, nb_g, blo, eng) in DRAINS:
                        if dpi != pi:
                            continue
                        if kind == "pair":
                            gi = col // 512
                            src = pt[0:M].rearrange(
                                "p (g r) -> p g r", g=4, r=512)[
                                :, gi:gi + 2, 0:nb_g * CH].rearrange(
                                "p g (o i) -> p o g i", o=CH, i=nb_g)
                            dst = ob_v[:, :, blo:blo + 2 * nb_g].rearrange(
                                "p o (g i) -> p o g i", g=2, i=nb_g)
                        else:
                            src = pt[0:M, col:col + nb_g * CH].rearrange(
                                "p (o i) -> p o i", o=CH, i=nb_g)
                            dst = ob_v[:, :, blo:blo + nb_g]
                        if eng == "act":
                            nc.scalar.copy(dst, src)
                        else:
                            nc.vector.tensor_copy(dst, src)
                if not skip_out:
                    nc.sync.dma_start(
                        z_d[s, t0:t0 + M].rearrange("p a b -> p (a b)"),
                        ob[0:M, :])

            for _rep in range(repeat):
                # interleaved emission: every engine queue is ordered by
                # expected data-ready time so in-order queues never block
                # early-ready work behind late-ready work.  DMA priority on
                # the sync queue: A(s0) -> p1 -> A(s1) -> out stream.
                A0 = loads(0)
                # tiny DMA depending on A0 keeps the p1/A1 transfers from
                # stealing bandwidth before sample-0's input has landed
                nc.sync.dma_start(stg[:], A0[2][0:1, 0:1])
                if _rep == 0:
                    nc.sync.dma_start(p1_sb[:], p1_d[:])
                A1 = loads(1)
                st0 = [stats_ft(0, A0, g) for g in range(3)]
                wt_build(0, weights(0, st0))
                chunk(0, *CHUNKS[0])
                st1 = [stats_ft(1, A1, 0)]
                chunk(0, *CHUNKS[1])
                st1.append(stats_ft(1, A1, 1))
                chunk(0, *CHUNKS[2])
                st1.append(stats_ft(1, A1, 2))
                chunk(0, *CHUNKS[3])
                csb1 = weights(1, st1)
                chunk(0, *CHUNKS[4])
                wt_build(1, csb1)
                for (t0, M) in CHUNKS[5:n_chunks]:
                    chunk(0, t0, M)
                for (t0, M) in CHUNKS[:n_chunks]:
                    chunk(1, t0, M)

    _NC_CACHE[key] = nc
    return nc


# ----------------------------------------------------------------------------
# Public entry point
# ----------------------------------------------------------------------------
def kernel(x, gn_w, gn_b, fc_w, fc_b):
    x = np.asarray(x, np.float32)
    gn_w = np.asarray(gn_w, np.float32)
    gn_b = np.asarray(gn_b, np.float32)
    fc_w = np.asarray(fc_w, np.float32)
    fc_b = np.asarray(fc_b, np.float32)

    p1, msel, ind, invct2 = _build_const_tables(gn_w, gn_b, fc_w, fc_b)
    np_dt = np.float16 if MM_DT == "f16" else np.float32
    ones16 = np.ones((14, B_LOC * N_XT * T), np_dt)
    p1 = p1.astype(np_dt)
    x = x.astype(np_dt)
    nc = build_bass()
    if not getattr(nc, "_waits_spilled", False):
        _spill_waits(nc)
        nc._waits_spilled = True

    in_maps = []
    for k in range(N_CORES):
        in_maps.append({
            "x": np.ascontiguousarray(x[k * B_LOC:(k + 1) * B_LOC]),
            "p1": p1, "msel": msel, "ind": ind,
            "invct2": invct2, "ones16": ones16,
        })
    res = run_bass_kernel_spmd(nc, in_maps, core_ids=list(range(N_CORES)))
    z = np.concatenate([r["z"] for r in res.results], axis=0)
    return z
